# revision 33
# baseline (speedup 1.0000x reference)
"""DWBNN (candidate-sampled 2-layer MLP) Trainium2 kernel, 8-core SPMD.

Math (per reference):
    Wsel[i,o] = weights[i,o,idx[i,o]]
    h   = relu(x @ Wsel1 + b1)        x:[B,D0] w1:[D0,D1,8]
    out = relu(h @ Wsel2 + b2)        w2:[D1,D2,8]     -> [B, D2]

Sharding: column-parallel layer 1 / row-parallel layer 2 over D1.
Core i holds D1-shard [1024*i, 1024*(i+1)): it computes hT_i = relu(x@W1sel)_i.T
locally, then partial_i = (h_i @ W2sel_i).T without any cross-core traffic.
A second small SPMD launch reduces the 8 partials (sum + bias2 + relu +
transpose), sharded over batch.

On-chip candidate select: weight tables are pre-arranged on host to
candidate-major [8, K, M] bf16; per K-block of 128 partitions the kernel
streams all 8 candidate slabs and reduces them with 8x (is_equal mask ->
copy_predicated) on DVE, producing the selected bf16 lhsT for the matmuls.
"""

import os
from contextlib import ExitStack

import ml_dtypes
import numpy as np

import concourse.bass as bass
import concourse.tile as tile
from concourse import bacc, mybir
from concourse.bass_utils import run_bass_kernel_spmd

BF16 = mybir.dt.bfloat16
F32 = mybir.dt.float32
I16 = mybir.dt.int16
AF = mybir.ActivationFunctionType
EQ = mybir.AluOpType.is_equal

NCORES = 8
WC = 8  # candidates per weight
KSPLIT = os.environ.get("DWBNN_KSPLIT", "0") == "1"

# Full-problem sizes (hardcoded per contract)
B_FULL, D0_FULL, D1_FULL, D2_FULL = 4096, 2048, 8192, 2048

# Results of the last device runs (for test harness inspection)
LAST = {}


# --------------------------------------------------------------------------
# device program: main launch (layers 1+2, per-core partial output)
# --------------------------------------------------------------------------

def _emit_select(nc, pools, wsel_tile, wr_ap, idx_ap, kb, hi, m0, mw):
    """Select candidates into wsel_tile[:, m0:m0+mw] for K-block kb.

    wr_ap: DRAM AP [nK, nH, 128, WC*mw] bf16 pre-packed on host in exact
    SBUF tile order, so each partition row is one contiguous 2*WC*mw-byte
    DMA packet.
    idx_ap: DRAM AP [K, M] int16.
    """
    w_t = pools["wraw"].tile([128, WC * mw], BF16, tag="wraw")
    nc.sync.dma_start(w_t[:], wr_ap[kb, hi])
    idx_t = pools["idx"].tile([128, mw], I16, tag="idx")
    nc.sync.dma_start(idx_t[:], idx_ap[kb * 128:(kb + 1) * 128, m0:m0 + mw])
    # candidate 0 unconditionally (4x-mode bf16 copy), then 7 predicated
    # overwrites: elements keep w_0 only where idx == 0.
    nc.vector.tensor_copy(wsel_tile[:, m0:m0 + mw], w_t[:, 0:mw])
    for c in range(1, WC):
        mask_t = pools["mask"].tile([128, mw], I16, tag="mask")
        nc.vector.tensor_scalar(mask_t[:], idx_t[:], c, None, op0=EQ)
        nc.vector.copy_predicated(
            wsel_tile[:, m0:m0 + mw], mask_t[:], w_t[:, c * mw:(c + 1) * mw]
        )


def _emit_layer(nc, pools, *, wr_ap, idx_ap, rhs_ap, out_cb, K, M, B, BQ, NMM,
                tag_prefix, sel_chunk=1024, ksplit=False, ident=None):
    """One dense layer: out[M, B] = Wsel[K, M].T @ rhs[K, B], tiled.

    out_cb(mb, bq, psum_tile) consumes each [128, BQ] fp32 psum block.

    ksplit: accumulate K in two halves. The first half's matmuls only
    need the first half of the selects, so the tensor engine starts
    ~2x earlier; half-sums round-trip DRAM as bf16 and are re-injected
    into the second half's PSUM group via an identity matmul.
    """
    nK, nM, nB = K // 128, M // 128, B // BQ

    # Phase A: candidate-select all K-blocks' weights.
    wsel = [
        pools["wsel"].tile([128, M], BF16, tag=f"wsel_{tag_prefix}_{kb}",
                           name=f"wsel_{tag_prefix}_{kb}")
        for kb in range(nK)
    ]
    for kb in range(nK):
        for hi, m0 in enumerate(range(0, M, sel_chunk)):
            _emit_select(nc, pools, wsel[kb], wr_ap, idx_ap, kb, hi, m0,
                         min(sel_chunk, M - m0))

    def rhs_tile(kb, bq):
        rt = pools["rhs"].tile([128, BQ], BF16, tag=f"rhs_{kb}")
        nc.sync.dma_start(
            rt[:], rhs_ap[kb * 128:(kb + 1) * 128, bq * BQ:(bq + 1) * BQ]
        )
        return rt

    def mm_sweep(ps, kbs, rhs_tiles, mb, first, last):
        for j, kb in enumerate(kbs):
            for n0 in range(0, BQ, NMM):
                nc.tensor.matmul(
                    ps[:, n0:n0 + NMM],
                    wsel[kb][:, mb * 128:(mb + 1) * 128],
                    rhs_tiles[j][:, n0:n0 + NMM],
                    start=first and (j == 0),
                    stop=last and (j == len(kbs) - 1),
                )

    if not ksplit:
        for bq in range(nB):
            rhs_tiles = [rhs_tile(kb, bq) for kb in range(nK)]
            for mb in range(nM):
                ps = pools["psum"].tile([128, BQ], F32, tag="ps")
                mm_sweep(ps, range(nK), rhs_tiles, mb, True, True)
                out_cb(mb, bq, ps)
        return

    hK = nK // 2
    hA = pools["dram"].tile([M, B], BF16, tag=f"hA_{tag_prefix}")
    # Group A: k-blocks [0, hK) for every (bq, mb) unit
    for bq in range(nB):
        rhs_tiles = [rhs_tile(kb, bq) for kb in range(hK)]
        for mb in range(nM):
            ps = pools["psum"].tile([128, BQ], F32, tag="ps")
            mm_sweep(ps, range(hK), rhs_tiles, mb, True, True)
            st = pools["hstage"].tile([128, BQ], BF16, tag="hastage")
            nc.scalar.activation(st[:], ps[:], AF.Copy)
            nc.sync.dma_start(
                hA[mb * 128:(mb + 1) * 128, bq * BQ:(bq + 1) * BQ], st[:]
            )
    # Group B: re-inject A-half sums, then k-blocks [hK, nK)
    for bq in range(nB):
        rhs_tiles = [rhs_tile(kb, bq) for kb in range(hK, nK)]
        for mb in range(nM):
            ha_t = pools["rhs"].tile([128, BQ], BF16, tag="ha_in")
            nc.sync.dma_start(
                ha_t[:], hA[mb * 128:(mb + 1) * 128, bq * BQ:(bq + 1) * BQ]
            )
            ps = pools["psum"].tile([128, BQ], F32, tag="ps")
            for n0 in range(0, BQ, NMM):
                nc.tensor.matmul(
                    ps[:, n0:n0 + NMM], ident[:], ha_t[:, n0:n0 + NMM],
                    start=True, stop=False,
                )
            mm_sweep(ps, range(hK, nK), rhs_tiles, mb, False, True)
            out_cb(mb, bq, ps)


def main_body(tc, outs, ins, cfg):
    """Launch-1 kernel body. ins/outs are dicts of DRAM APs."""
    nc = tc.nc
    B, D0, D1S, D2 = cfg["B"], cfg["D0"], cfg["D1S"], cfg["D2"]
    BQ, NMM = cfg["BQ"], cfg["NMM"]

    with ExitStack() as ctx:
        pools = {}
        for name, bufs, space in [
            ("wsel", 1, "SBUF"), ("wraw", 3, "SBUF"), ("idx", 3, "SBUF"),
            ("mask", 2, "SBUF"), ("rhs", 1, "SBUF"), ("psum", 4, "PSUM"),
            ("hstage", 3, "SBUF"), ("postage", 3, "SBUF"), ("bias", 1, "SBUF"),
            ("dram", 1, "DRAM"),
        ]:
            pools[name] = ctx.enter_context(
                tc.tile_pool(name=name, bufs=bufs, space=space)
            )

        # bias1 tiles: [128, 1] per output block of layer 1
        nM1 = D1S // 128
        b1_t = []
        for mb in range(nM1):
            bt = pools["bias"].tile([128, 1], F32, tag=f"b1_{mb}")
            nc.sync.dma_start(bt[:], ins["b1"][mb * 128:(mb + 1) * 128, :])
            b1_t.append(bt)

        # hT scratch in DRAM: [D1S, B] bf16
        hT = pools["dram"].tile([D1S, B], BF16, tag="hT")

        from concourse.masks import make_identity
        ident = pools["bias"].tile([128, 128], BF16, tag="ident")
        make_identity(nc, ident[:])

        def l1_out(mb, bq, ps):
            hs = pools["hstage"].tile([128, BQ], BF16, tag="hstage")
            nc.scalar.activation(hs[:], ps[:], AF.Relu, bias=b1_t[mb][:, 0:1])
            nc.sync.dma_start(
                hT[mb * 128:(mb + 1) * 128, bq * BQ:(bq + 1) * BQ], hs[:]
            )

        _emit_layer(
            nc, pools, wr_ap=ins["w1"], idx_ap=ins["idx1"], rhs_ap=ins["xT"],
            out_cb=l1_out, K=D0, M=D1S, B=B, BQ=BQ, NMM=NMM, tag_prefix="l1",
            ksplit=KSPLIT, ident=ident,
        )

        def l2_out(mb, bq, ps):
            po = pools["postage"].tile([128, BQ], F32, tag="postage")
            nc.scalar.activation(po[:], ps[:], AF.Copy)
            nc.sync.dma_start(
                outs["pout"][mb * 128:(mb + 1) * 128, bq * BQ:(bq + 1) * BQ], po[:]
            )

        _emit_layer(
            nc, pools, wr_ap=ins["w2"], idx_ap=ins["idx2"], rhs_ap=hT,
            out_cb=l2_out, K=D1S, M=D2, B=B, BQ=BQ, NMM=NMM, tag_prefix="l2",
        )


# --------------------------------------------------------------------------
# device program: reduce launch (sum partials + bias2 + relu + transpose)
# --------------------------------------------------------------------------

def reduce_body(tc, outs, ins, cfg):
    """ins: pstack [D2//128, 128, P*BS] f32 (host-packed so each partition
    row is contiguous), b2 [D2, 1] f32 -> outs: oslice [BS, D2] f32."""
    nc = tc.nc
    P, D2, BS = cfg["P"], cfg["D2"], cfg["BS"]
    from concourse.masks import make_identity

    with ExitStack() as ctx:
        pool = ctx.enter_context(tc.tile_pool(name="sb", bufs=5))
        const = ctx.enter_context(tc.tile_pool(name="const", bufs=1))
        psum = ctx.enter_context(tc.tile_pool(name="ps", bufs=8, space="PSUM"))

        ident = const.tile([128, 128], F32, tag="ident")
        make_identity(nc, ident[:])

        nD = D2 // 128
        b2_t = []
        for mb in range(nD):
            bt = const.tile([128, 1], F32, tag=f"b2_{mb}")
            nc.sync.dma_start(bt[:], ins["b2"][mb * 128:(mb + 1) * 128, :])
            b2_t.append(bt)
        owide = [
            const.tile([128, D2], F32, tag=f"ow_{tb}", name=f"ow_{tb}")
            for tb in range(BS // 128)
        ]

        for mb in range(nD):
            # one DMA pulls this o2-block from all P partials (host-packed)
            pt = pool.tile([128, P * BS], F32, tag="pt")
            nc.sync.dma_start(pt[:], ins["pstack"][mb])
            acc = pool.tile([128, BS], F32, tag="acc")
            nc.vector.tensor_add(acc[:], pt[:, 0:BS], pt[:, BS:2 * BS])
            for i in range(2, P):
                nc.vector.tensor_add(acc[:], acc[:], pt[:, i * BS:(i + 1) * BS])
            rl = pool.tile([128, BS], F32, tag="rl")
            nc.scalar.activation(rl[:], acc[:], AF.Relu, bias=b2_t[mb][:, 0:1])
            for tb in range(BS // 128):
                pst = psum.tile([128, 128], F32, tag="pst")
                nc.tensor.transpose(pst[:], rl[:, tb * 128:(tb + 1) * 128], ident[:])
                nc.vector.tensor_copy(
                    owide[tb][:, mb * 128:(mb + 1) * 128], pst[:]
                )
        # wide output rows -> one 8KB-run DMA per batch block
        for tb in range(BS // 128):
            nc.sync.dma_start(
                outs["oslice"][tb * 128:(tb + 1) * 128, :], owide[tb][:]
            )


# --------------------------------------------------------------------------
# program builders
# --------------------------------------------------------------------------

def _w_dram_shape(K, M, chunk=1024):
    mw = min(chunk, M)
    return [K // 128, (M + mw - 1) // mw, 128, WC * mw]


def pack_w(wb, chunk=1024):
    """[K, M, WC] -> [nK, nH, 128, WC*mw] in exact SBUF tile order."""
    K, M, _ = wb.shape
    mw = min(chunk, M)
    nK, nH = K // 128, M // mw
    return np.ascontiguousarray(
        wb.reshape(nK, 128, nH, mw, WC).transpose(0, 2, 1, 4, 3)
    ).reshape(nK, nH, 128, WC * mw)


def build_main(cfg):
    nc = bacc.Bacc("TRN2", target_bir_lowering=False, debug=False,
                   num_devices=cfg["n_cores"])
    B, D0, D1S, D2 = cfg["B"], cfg["D0"], cfg["D1S"], cfg["D2"]
    ins = {
        "xT": nc.dram_tensor("xT", [D0, B], BF16, kind="ExternalInput").ap(),
        "w1": nc.dram_tensor("w1", _w_dram_shape(D0, D1S), BF16,
                             kind="ExternalInput").ap(),
        "idx1": nc.dram_tensor("idx1", [D0, D1S], I16, kind="ExternalInput").ap(),
        "b1": nc.dram_tensor("b1", [D1S, 1], F32, kind="ExternalInput").ap(),
        "w2": nc.dram_tensor("w2", _w_dram_shape(D1S, D2), BF16,
                             kind="ExternalInput").ap(),
        "idx2": nc.dram_tensor("idx2", [D1S, D2], I16, kind="ExternalInput").ap(),
    }
    outs = {
        "pout": nc.dram_tensor("pout", [D2, B], F32, kind="ExternalOutput").ap(),
    }
    with tile.TileContext(nc) as tc:
        main_body(tc, outs, ins, cfg)
    nc.compile()
    return nc


def build_reduce(cfg):
    nc = bacc.Bacc("TRN2", target_bir_lowering=False, debug=False,
                   num_devices=cfg["n_cores"])
    P, D2, BS = cfg["P"], cfg["D2"], cfg["BS"]
    ins = {
        "pstack": nc.dram_tensor("pstack", [D2 // 128, 128, P * BS], F32,
                                 kind="ExternalInput").ap(),
        "b2": nc.dram_tensor("b2", [D2, 1], F32, kind="ExternalInput").ap(),
    }
    outs = {
        "oslice": nc.dram_tensor("oslice", [BS, D2], F32, kind="ExternalOutput").ap(),
    }
    with tile.TileContext(nc) as tc:
        reduce_body(tc, outs, ins, cfg)
    nc.compile()
    return nc


# --------------------------------------------------------------------------
# host entry point
# --------------------------------------------------------------------------

def kernel(x, weights1, bias1, idx1, weights2, bias2, idx2):
    x = np.asarray(x, dtype=np.float32)
    weights1 = np.asarray(weights1)
    bias1 = np.asarray(bias1, dtype=np.float32)
    idx1 = np.asarray(idx1)
    weights2 = np.asarray(weights2)
    bias2 = np.asarray(bias2, dtype=np.float32)
    idx2 = np.asarray(idx2)

    B, D0 = x.shape
    D1 = weights1.shape[1]
    D2 = weights2.shape[1]
    assert (B, D0, D1, D2) == (B_FULL, D0_FULL, D1_FULL, D2_FULL)
    D1S = D1 // NCORES
    BS = B // NCORES

    bf16 = ml_dtypes.bfloat16
    cfg_main = dict(B=B, D0=D0, D1S=D1S, D2=D2, BQ=1024, NMM=512, n_cores=NCORES)
    cfg_red = dict(P=NCORES, D2=D2, BS=BS, n_cores=NCORES)

    # ---- host-side input staging ----
    xT = np.ascontiguousarray(x.T).astype(bf16)  # [D0, B]
    w1b = weights1.astype(bf16)
    w2b = weights2.astype(bf16)
    i1 = idx1.astype(np.int16)
    i2 = idx2.astype(np.int16)

    in_maps = []
    for ci in range(NCORES):
        sl = slice(ci * D1S, (ci + 1) * D1S)
        in_maps.append({
            "xT": xT,
            "w1": pack_w(w1b[:, sl, :]),
            "idx1": np.ascontiguousarray(i1[:, sl]),
            "b1": np.ascontiguousarray(bias1[sl])[:, None],
            "w2": pack_w(w2b[sl, :, :]),
            "idx2": np.ascontiguousarray(i2[sl, :]),
        })

    nc_main = build_main(cfg_main)
    res_main = run_bass_kernel_spmd(
        nc_main, in_maps, core_ids=list(range(NCORES)), trace=False
    )
    LAST["main"] = res_main
    partials = [res_main.results[ci]["pout"] for ci in range(NCORES)]

    # ---- reduce launch: shard over batch ----
    nc_red = build_reduce(cfg_red)
    b2c = np.ascontiguousarray(bias2)[:, None]
    red_maps = []
    for cj in range(NCORES):
        bsl = slice(cj * BS, (cj + 1) * BS)
        # [P, D2, BS] -> [D2//128, 128, P, BS] packed: contiguous partition rows
        stk = np.stack([p[:, bsl] for p in partials], axis=0)
        packed = np.ascontiguousarray(
            stk.transpose(1, 0, 2).reshape(D2 // 128, 128, NCORES * BS)
        )
        red_maps.append({"pstack": packed, "b2": b2c})
    res_red = run_bass_kernel_spmd(
        nc_red, red_maps, core_ids=list(range(NCORES)), trace=False
    )
    LAST["reduce"] = res_red

    out = np.concatenate(
        [res_red.results[cj]["oslice"] for cj in range(NCORES)], axis=0
    )
    return np.ascontiguousarray(out.astype(np.float32))
